# revision 34
# baseline (speedup 1.0000x reference)
# CrossAttention (B=2, S=2048, D=1024, H=16, dh=64) on 8 trn2 NeuronCores.
#
# Sharding: 32 (batch, head) units, 4 consecutive units per core (cores 0-3
# work on batch 0, cores 4-7 on batch 1). Each core receives its batch's
# hidden states pre-permuted to the on-chip [128, D/128, S] transposed
# layout, per-head slices of Wq/Wk/Wv/Wo (also pre-permuted); it returns a
# partial output y [2048, 1024] (its heads' contribution to the output
# projection). The host sums the four partials per batch and adds bo.
#
# Device algorithm (per core, 4 heads = 2 pairs). The cost model prices a
# matmul at (moving free size) x (cycles/row): fp32r is 1 cyc/row only at
# N>=256, bf16 is 1 cyc/row at any N, and M/K are free. The kernel exploits
# this by keeping the big-N matmuls (projections, scores) in fp32r and
# running PV "swapped" with tiny-N bf16 moving operands:
#   - projections: Q^T/K^T pair-packed [128, s] fp32r (N=512); V in natural
#     [s, feat] layout converted to bf16 with a fused ones column per head.
#   - scores: S^T = K^T-chunk^T Q^T per pair (K=64, M=128 keys, N=512).
#   - P^T = exp(S^T/8) on ACT, written as bf16 (PV stationary).
#   - PV swapped: stationary = P^T tile [128 keys, 128 q], moving =
#     [V_h | 1] bf16 [128 keys, 65] -> psum [128 q, 65] per head; one psum
#     accumulation group per (cell, qtile) over 4 key chunks x 4 heads;
#     column 64 accumulates the softmax denominator. Half the PE cycles of
#     the classic V'^T P^T orientation, and O lands in [q, feat] layout.
#   - normalize: DVE reciprocal of psum col 64, per-partition
#     tensor_scalar_mul -> bf16 O tiles; PE bf16 transpose (identity
#     permutation) -> O^T [feat, q]; output projection with K=128 (head
#     pairs packed) and fp32r moving Wo^T (N=512).
# Triangular software pipeline over 512-wide s-block rounds keeps the ACT
# engine (exp is ~equal to total PE work) fed from round 0.
import os
import sys

import numpy as np

try:
    import concourse.bass as bass
except ImportError:  # harness runs from a fresh dir; repo is on the default path
    sys.path.insert(0, "/opt/trn_rl_repo")
    import concourse.bass as bass

import concourse.bacc as bacc
import concourse.mybir as mybir
import concourse.tile as tile
from concourse.bass import ts, ds
from contextlib import ExitStack

B, S, D = 2, 2048, 1024
HEADS, DIM_HEAD = 16, 64
SCALE = DIM_HEAD**-0.5
N_CORES = 8
UNITS = 4  # heads per core
PAIRS = 2  # head pairs per core
P = 128
SB = S // 512  # 4 s-block rounds (key blocks)
QBN = S // 512  # 4 q-blocks
DC = D // P  # 8 contraction chunks for projections
KI = S // P  # 16 key chunks of 128
F32 = mybir.dt.float32
F32R = mybir.dt.float32r
BF16 = mybir.dt.bfloat16


def build_nc():
    nc = bacc.Bacc("TRN2", target_bir_lowering=False, debug=False)

    hiddent = nc.dram_tensor("hiddent", [P, DC, S], BF16, kind="ExternalInput").ap()
    wqt = nc.dram_tensor("wqt", [P, DC, 256], BF16, kind="ExternalInput").ap()
    wkt = nc.dram_tensor("wkt", [P, DC, 256], BF16, kind="ExternalInput").ap()
    wvt = nc.dram_tensor("wvt", [P, DC, 256], BF16, kind="ExternalInput").ap()
    # Wo^T pair-packed: wot[f, pair, d] = Wo[d, f0 + pair*128 + f]; bf16 so
    # the O-projection (bf16 O^T stationary) has matching input dtypes
    wot = nc.dram_tensor("wot", [P, PAIRS, D], BF16, kind="ExternalInput").ap()
    y = nc.dram_tensor("y", [S, D], F32, kind="ExternalOutput").ap()

    K_PT = int(os.environ.get("K_PT", "28"))
    K_ST = int(os.environ.get("K_ST", "2"))
    K_CELL = int(os.environ.get("K_CELL", "1"))
    K_FIN = int(os.environ.get("K_FIN", "3"))
    K_DEFER = int(os.environ.get("K_DEFER", "0"))
    K_LASTFIRST = os.environ.get("K_LASTFIRST", "0") == "1"

    with tile.TileContext(nc) as tc, ExitStack() as ctx:
        persist = ctx.enter_context(tc.tile_pool(name="persist", bufs=1))
        pt_pool = ctx.enter_context(tc.tile_pool(name="pt", bufs=K_PT))
        otu_pool = ctx.enter_context(tc.tile_pool(name="otu", bufs=4))
        ott_pool = ctx.enter_context(tc.tile_pool(name="ott", bufs=4))
        rc_pool = ctx.enter_context(tc.tile_pool(name="rc", bufs=2))
        y_pool = ctx.enter_context(tc.tile_pool(name="ysb", bufs=2))
        ht_pool = ctx.enter_context(tc.tile_pool(name="htp", bufs=2))
        # PSUM (8 banks): S^T tiles 2x[128,1024] (4 banks), PV accumulators
        # 2x[128,4x65] (2 banks), projection/transpose/output transients
        # 2x[128,512] (2 banks).
        st_ps = ctx.enter_context(
            tc.tile_pool(name="stps", bufs=K_ST, space="PSUM")
        )
        cell_ps = ctx.enter_context(
            tc.tile_pool(name="cellps", bufs=K_CELL, space="PSUM")
        )
        fin_ps = ctx.enter_context(
            tc.tile_pool(name="finps", bufs=K_FIN, space="PSUM")
        )

        # ---- persistent SBUF tensors ----
        KT = persist.tile([P, PAIRS, S], F32R)  # K^T pair-packed
        QT = persist.tile([P, PAIRS, S], F32R)  # Q^T pair-packed
        # V natural layout per (k-chunk, head): [V_h(64) | 1] in bf16
        Vp = persist.tile([P, KI, UNITS, 65], BF16)
        wq_sb = persist.tile([P, DC, 256], BF16)
        wk_sb = persist.tile([P, DC, 256], BF16)
        wv_a = persist.tile([P, 4, 256], BF16)
        wv_b = persist.tile([P, 4, 256], BF16)
        wo_sb = persist.tile([P, PAIRS, D], BF16)  # Wo^T pair-packed (K=128)
        ones_f32 = persist.tile([P, P], F32)
        ident = persist.tile([P, P], BF16)  # identity for PE transpose
        # O accumulator: [q-tile partitions, qtile, head, 64 feats + denom]
        acc = persist.tile([P, KI, UNITS, 65], F32)

        nc.vector.memset(ones_f32, 1.0)
        # identity: keep 1.0 where partition == column, else 0
        ones_bf = persist.tile([P, P], BF16)
        nc.vector.tensor_copy(ones_bf, ones_f32)
        nc.gpsimd.affine_select(
            ident,
            ones_bf,
            pattern=[[-1, P]],
            compare_op=mybir.AluOpType.is_equal,
            fill=0.0,
            base=0,
            channel_multiplier=1,
        )
        # ones columns of V' (col 64 per head); V writes only cols 0:64
        nc.vector.memset(Vp[:, :, :, 64:65], 1.0)
        # warm the ACT Exp table before the first real exp
        warm = persist.tile([P, 1], BF16)
        nc.scalar.activation(
            warm, ones_f32[:, 0:1], mybir.ActivationFunctionType.Exp, scale=1.0
        )
        # warm the PE p-state (2.4GHz needs ~3us of continuous execution):
        # junk matmuls on ones keep the PE busy while the first weight and
        # hidden DMAs land, so the real projections start at full clock
        for w in range(int(os.environ.get("K_WARM", "10"))):
            wps = fin_ps.tile([P, 64], F32, tag="fin", name="wps")
            nc.tensor.matmul(
                wps, ones_f32, ones_f32[:, 0:64], start=True, stop=True
            )
        # first-round weights split across both DMA queues so neither
        # head-blocks: K on HWDGE before the hidden quarters, Q first on
        # SWDGE; V/Wo are emitted after round 0's hidden quarters
        nc.sync.dma_start(wk_sb, wkt)
        nc.gpsimd.dma_start(wq_sb, wqt)

        # ---- cell phases ----
        pv_queue = []  # (qb, kc, pts) with exps emitted, PV pending

        def s_phase(qb, kc, p):
            """S^T + exp for one head pair of cell (qb, kc); returns pt tiles."""
            pts = []
            for k4 in range(4):
                ki = kc * 4 + k4
                stt = st_ps.tile([P, 1024], F32, tag="st")
                nc.tensor.matmul(
                    stt[:, 0:512],
                    KT[0:64, p, ts(ki, P)],
                    QT[0:64, p, ts(qb, 512)],
                    start=True,
                    stop=True,
                )
                nc.tensor.matmul(
                    stt[:, 512:1024],
                    KT[64:128, p, ts(ki, P)],
                    QT[64:128, p, ts(qb, 512)],
                    start=True,
                    stop=True,
                )
                pt = pt_pool.tile([P, 1024], BF16)
                nc.scalar.activation(
                    pt, stt, mybir.ActivationFunctionType.Exp, scale=SCALE
                )
                pts.append(pt)
            return pts

        def pv_phase(qb, kc, pts, y_on_act=False):
            """PV for cell (qb, kc): per q-tile, one psum accumulation group
            over 4 heads x 4 key chunks; accumulate into acc on DVE. On the
            final cell of a q-block, finalize each q-tile as soon as its
            accumulation lands (shortens the post-exp tail)."""
            for qt_i in range(4):
                if kc == SB - 1 and os.environ.get("K_CELLFIN", "0") == "1":
                    # final cells share the fin ring (3 bufs): their PV,
                    # transpose, and output-projection psum pipelines
                    cp = fin_ps.tile([P, UNITS, 65], F32, tag="fin", name="cpf")
                else:
                    cp = cell_ps.tile([P, UNITS, 65], F32, tag="cell")
                n = 0
                for h in range(UNITS):
                    pr, hh = divmod(h, 2)
                    for k4 in range(4):
                        nc.tensor.matmul(
                            cp[:, h, :],
                            pts[pr][k4][:, hh * 512 + qt_i * P : hh * 512 + (qt_i + 1) * P],
                            Vp[:, kc * 4 + k4, h, :],
                            start=(n == 0),
                            stop=(n == 15),
                        )
                        n += 1
                sl = acc[:, qb * 4 + qt_i, :, :]
                if kc == 0:
                    nc.vector.tensor_copy(sl, cp)
                else:
                    nc.vector.tensor_add(sl, sl, cp)
                if kc == SB - 1:
                    finalize_qt(qb * 4 + qt_i, y_on_act)

        def finalize_qt(qt, y_on_act=False):
            """Normalize q-tile qt, transpose O, output projection, DMA."""
            rc = rc_pool.tile([P, UNITS], F32)
            nc.vector.reciprocal(
                rc, acc[:, qt, :, 64:65].rearrange("p h one -> p (h one)")
            )
            otts = []
            for pr in range(PAIRS):
                otu = otu_pool.tile([P, P], BF16)
                for hh in range(2):
                    h = 2 * pr + hh
                    # all-SBUF op: runs on the otherwise-idle GPSIMD
                    nc.gpsimd.tensor_scalar_mul(
                        otu[:, hh * 64 : (hh + 1) * 64],
                        acc[:, qt, h, 0:64],
                        rc[:, h : h + 1],
                    )
                tp = fin_ps.tile([P, P], BF16, tag="fin", name="tp")
                nc.tensor.transpose(tp, otu, ident)
                ott = ott_pool.tile([P, P], BF16)
                nc.vector.tensor_copy(ott, tp)
                otts.append(ott)
            for dh in range(2):
                yps = fin_ps.tile([P, 512], F32, tag="fin", name="yps")
                for pr in range(PAIRS):
                    nc.tensor.matmul(
                        yps,
                        otts[pr],
                        wo_sb[:, pr, ds(dh * 512, 512)],
                        start=(pr == 0),
                        stop=(pr == PAIRS - 1),
                    )
                ysb = y_pool.tile([P, 512], F32)
                # the tail finalize copies y on ACT (idle once exps drain);
                # earlier ones stay on DVE to keep ACT on the exp stream
                if y_on_act:
                    nc.scalar.copy(ysb, yps)
                else:
                    nc.vector.tensor_copy(ysb, yps)
                nc.sync.dma_start(
                    y[qt * P : (qt + 1) * P, ds(dh * 512, 512)], ysb
                )

        def emit_s(qb, kc):
            pts = [s_phase(qb, kc, 0), s_phase(qb, kc, 1)]
            pv_queue.append((qb, kc, pts))

        n_final = [0]

        def drain_pv(keep, max_kc=SB - 1):
            """Emit pending PVs. max_kc gates cells whose kc-block V
            projection hasn't been emitted yet (tile deps are emission-
            ordered: a PV emitted before its V write would read stale Vp)."""
            while len(pv_queue) > keep and pv_queue[0][1] <= max_kc:
                qb, kc, pts = pv_queue.pop(0)
                if kc == SB - 1:
                    n_final[0] += 1
                pv_phase(
                    qb,
                    kc,
                    pts,
                    y_on_act=(
                        os.environ.get("K_YACT", "0") == "1"
                        and kc == SB - 1
                        and n_final[0] >= QBN
                    ),
                )

        # ---- triangular pipeline ----
        # hidden DMAs and K/Q projections run one round ahead of the cell
        # stream, so each round's cells unlock with no ACT gap; V
        # projections trail the first cell's S-phase (PV needs them ~8us
        # later).
        ht_tiles = {}

        def issue_dma(sb):
            hTa = ht_pool.tile([P, 4, 512], BF16, tag="hta", name="hTa")
            hTb = ht_pool.tile([P, 4, 512], BF16, tag="htb", name="hTb")
            if sb == 0:
                # split the cold-start load into quarters on both queues so
                # the first projection chunks begin as early as possible
                for dq in range(4):
                    nc.sync.dma_start(hTa[:, dq, :], hiddent[:, dq, ts(sb, 512)])
                    nc.gpsimd.dma_start(
                        hTb[:, dq, :], hiddent[:, 4 + dq, ts(sb, 512)]
                    )
                # V/Wo weights after the round-0 hidden (needed ~10us in)
                nc.sync.dma_start(wv_a, wvt[:, 0:4, :])
                nc.sync.dma_start(wv_b, wvt[:, 4:8, :])
                nc.sync.dma_start(wo_sb, wot)
            else:
                nc.sync.dma_start(hTa, hiddent[:, 0:4, ts(sb, 512)])
                nc.gpsimd.dma_start(hTb, hiddent[:, 4:8, ts(sb, 512)])
            ht_tiles[sb] = (hTa, hTb)

        def hts_of(sb):
            hTa, hTb = ht_tiles[sb]
            return lambda dc: (hTa if dc < 4 else hTb)[:, dc % 4, :]

        def emit_kq(sb, cell=None):
            """K/Q projections for s-block sb, pair-major. When `cell` is
            given (the first cell of round sb), each pair's S-phase is
            emitted right after that pair's two combos, so its exps keep
            ACT fed while the remaining combos run on PE."""
            hts = hts_of(sb)
            pts = []
            for p in range(PAIRS):
                for w_sb, out_t in ((wk_sb, KT), (wq_sb, QT)):
                    kps = fin_ps.tile([P, 512], F32, tag="fin", name="kps")
                    for dc in range(DC):
                        nc.tensor.matmul(
                            kps,
                            w_sb[:, dc, ts(p, P)],
                            hts(dc),
                            start=(dc == 0),
                            stop=(dc == DC - 1),
                        )
                    nc.vector.tensor_copy(out_t[:, p, ts(sb, 512)], kps)
                if cell is not None:
                    pts.append(s_phase(cell[0], cell[1], p))
            if cell is not None:
                pv_queue.append((cell[0], cell[1], pts))

        def emit_vproj(sb, st_i):
            hts = hts_of(sb)
            ki = sb * 4 + st_i
            vps = fin_ps.tile([P, 512], F32, tag="fin", name="vps")
            for dc in range(DC):
                nc.tensor.matmul(
                    vps[:, 0:256],
                    hts(dc)[:, ts(st_i, P)],
                    (wv_a if dc < 4 else wv_b)[:, dc % 4, :],
                    start=(dc == 0),
                    stop=(dc == DC - 1),
                )
            nc.vector.tensor_copy(
                Vp[:, ki, :, 0:64],
                vps[:, 0:256].rearrange("p (h f) -> p h f", h=UNITS),
            )

        issue_dma(0)
        # round 0's K/Q fused with cell (0,0)'s S-phases
        emit_kq(0, cell=(0, 0))
        for sb in range(SB):
            if sb + 1 < SB:
                issue_dma(sb + 1)
            # newly-ready cells: earlier q-blocks against this round's keys,
            # plus this q-block against all keys so far. Cell (0, sb) was
            # already emitted fused into emit_kq(sb).
            new_cells = [(qb, sb) for qb in range(1, sb)]
            new_cells += [(sb, kc) for kc in range(sb + 1)]
            if sb == 0:
                new_cells = []  # (0, 0) already emitted fused in emit_kq(0)
            for st_i in range(4):
                emit_vproj(sb, st_i)
            drain_pv(keep=1)
            for cell in new_cells:
                emit_s(*cell)
                drain_pv(keep=1)
            # next round's K/Q (+ its first cell) at the round's lowest
            # priority: PE falls back to them when the cell stream stalls
            if sb + 1 < SB:
                emit_kq(sb + 1, cell=(0, sb + 1))
        drain_pv(keep=0)
    nc.compile()
    return nc


_NC = None


def get_nc():
    global _NC
    if _NC is None:
        _NC = build_nc()
    return _NC


def shard_inputs(hidden_states, Wq, Wk, Wv, Wo):
    """Per-core input maps. Core c: batch c//4, heads 4*(c%4) .. 4*(c%4)+3."""
    import ml_dtypes

    hidden_states = np.asarray(hidden_states, np.float32)
    Wq, Wk, Wv, Wo = (np.asarray(w, np.float32) for w in (Wq, Wk, Wv, Wo))
    in_maps = []
    for c in range(N_CORES):
        b = c // 4
        f0 = (c % 4) * 4 * DIM_HEAD  # first feature row/col of this core's heads
        rows = slice(f0, f0 + UNITS * DIM_HEAD)

        def proj_layout(w):
            # W[rows].T is [D, 256]; on-chip layout is [128, DC, 256] bf16
            return np.ascontiguousarray(
                w[rows, :].T.reshape(DC, P, 256).transpose(1, 0, 2)
            ).astype(ml_dtypes.bfloat16)

        # Wo[:, rows].T is [256, D]; on-chip layout is [128, PAIRS, D] bf16
        wot = np.ascontiguousarray(
            Wo[:, rows].T.reshape(PAIRS, P, D).transpose(1, 0, 2)
        ).astype(ml_dtypes.bfloat16)
        in_maps.append(
            {
                "hiddent": np.ascontiguousarray(
                    hidden_states[b].T.reshape(DC, P, S).transpose(1, 0, 2)
                ).astype(ml_dtypes.bfloat16),
                "wqt": proj_layout(Wq),
                "wkt": proj_layout(Wk),
                "wvt": proj_layout(Wv),
                "wot": wot,
            }
        )
    return in_maps


def unshard_outputs(results, bo):
    out = np.zeros((B, S, D), np.float32)
    for c, res in enumerate(results):
        out[c // 4] += res["y"]
    out += np.asarray(bo, np.float32)[None, None, :]
    return out


def kernel(hidden_states, Wq, Wk, Wv, Wo, bo, _trace=False):
    from concourse.bass_utils import run_bass_kernel_spmd

    nc = get_nc()
    in_maps = shard_inputs(hidden_states, Wq, Wk, Wv, Wo)
    res = run_bass_kernel_spmd(nc, in_maps, list(range(N_CORES)), trace=_trace)
    out = unshard_outputs(res.results, bo)
    if _trace:
        return out, res
    return out


# revision 35
# speedup vs baseline: 1.0078x; 1.0078x over previous
# CrossAttention (B=2, S=2048, D=1024, H=16, dh=64) on 8 trn2 NeuronCores.
#
# Sharding: 32 (batch, head) units, 4 consecutive units per core (cores 0-3
# work on batch 0, cores 4-7 on batch 1). Each core receives its batch's
# hidden states pre-permuted to the on-chip [128, D/128, S] transposed
# layout, per-head slices of Wq/Wk/Wv/Wo (also pre-permuted); it returns a
# partial output y [2048, 1024] (its heads' contribution to the output
# projection). The host sums the four partials per batch and adds bo.
#
# Device algorithm (per core, 4 heads = 2 pairs). The cost model prices a
# matmul at (moving free size) x (cycles/row): fp32r is 1 cyc/row only at
# N>=256, bf16 is 1 cyc/row at any N, and M/K are free. The kernel exploits
# this by keeping the big-N matmuls (projections, scores) in fp32r and
# running PV "swapped" with tiny-N bf16 moving operands:
#   - projections: Q^T/K^T pair-packed [128, s] fp32r (N=512); V in natural
#     [s, feat] layout converted to bf16 with a fused ones column per head.
#   - scores: S^T = K^T-chunk^T Q^T per pair (K=64, M=128 keys, N=512).
#   - P^T = exp(S^T/8) on ACT, written as bf16 (PV stationary).
#   - PV swapped: stationary = P^T tile [128 keys, 128 q], moving =
#     [V_h | 1] bf16 [128 keys, 65] -> psum [128 q, 65] per head; one psum
#     accumulation group per (cell, qtile) over 4 key chunks x 4 heads;
#     column 64 accumulates the softmax denominator. Half the PE cycles of
#     the classic V'^T P^T orientation, and O lands in [q, feat] layout.
#   - normalize: DVE reciprocal of psum col 64, per-partition
#     tensor_scalar_mul -> bf16 O tiles; PE bf16 transpose (identity
#     permutation) -> O^T [feat, q]; output projection with K=128 (head
#     pairs packed) and fp32r moving Wo^T (N=512).
# Triangular software pipeline over 512-wide s-block rounds keeps the ACT
# engine (exp is ~equal to total PE work) fed from round 0.
import os
import sys

import numpy as np

try:
    import concourse.bass as bass
except ImportError:  # harness runs from a fresh dir; repo is on the default path
    sys.path.insert(0, "/opt/trn_rl_repo")
    import concourse.bass as bass

import concourse.bacc as bacc
import concourse.mybir as mybir
import concourse.tile as tile
from concourse.bass import ts, ds
from contextlib import ExitStack

B, S, D = 2, 2048, 1024
HEADS, DIM_HEAD = 16, 64
SCALE = DIM_HEAD**-0.5
N_CORES = 8
UNITS = 4  # heads per core
PAIRS = 2  # head pairs per core
P = 128
SB = S // 512  # 4 s-block rounds (key blocks)
QBN = S // 512  # 4 q-blocks
DC = D // P  # 8 contraction chunks for projections
KI = S // P  # 16 key chunks of 128
F32 = mybir.dt.float32
F32R = mybir.dt.float32r
BF16 = mybir.dt.bfloat16


def build_nc():
    nc = bacc.Bacc("TRN2", target_bir_lowering=False, debug=False)

    hiddent = nc.dram_tensor("hiddent", [P, DC, S], BF16, kind="ExternalInput").ap()
    wqt = nc.dram_tensor("wqt", [P, DC, 256], BF16, kind="ExternalInput").ap()
    wkt = nc.dram_tensor("wkt", [P, DC, 256], BF16, kind="ExternalInput").ap()
    wvt = nc.dram_tensor("wvt", [P, DC, 256], BF16, kind="ExternalInput").ap()
    # Wo^T pair-packed: wot[f, pair, d] = Wo[d, f0 + pair*128 + f]; bf16 so
    # the O-projection (bf16 O^T stationary) has matching input dtypes
    wot = nc.dram_tensor("wot", [P, PAIRS, D], BF16, kind="ExternalInput").ap()
    y = nc.dram_tensor("y", [S, D], F32, kind="ExternalOutput").ap()

    K_PT = int(os.environ.get("K_PT", "28"))
    K_ST = int(os.environ.get("K_ST", "2"))
    K_CELL = int(os.environ.get("K_CELL", "1"))
    K_FIN = int(os.environ.get("K_FIN", "3"))
    K_DEFER = int(os.environ.get("K_DEFER", "0"))
    K_LASTFIRST = os.environ.get("K_LASTFIRST", "0") == "1"

    with tile.TileContext(nc) as tc, ExitStack() as ctx:
        persist = ctx.enter_context(tc.tile_pool(name="persist", bufs=1))
        pt_pool = ctx.enter_context(tc.tile_pool(name="pt", bufs=K_PT))
        otu_pool = ctx.enter_context(tc.tile_pool(name="otu", bufs=6))
        ott_pool = ctx.enter_context(tc.tile_pool(name="ott", bufs=6))
        rc_pool = ctx.enter_context(tc.tile_pool(name="rc", bufs=4))
        y_pool = ctx.enter_context(tc.tile_pool(name="ysb", bufs=4))
        ht_pool = ctx.enter_context(tc.tile_pool(name="htp", bufs=2))
        # PSUM (8 banks): S^T tiles 2x[128,1024] (4 banks), PV accumulators
        # 2x[128,4x65] (2 banks), projection/transpose/output transients
        # 2x[128,512] (2 banks).
        st_ps = ctx.enter_context(
            tc.tile_pool(name="stps", bufs=K_ST, space="PSUM")
        )
        cell_ps = ctx.enter_context(
            tc.tile_pool(name="cellps", bufs=K_CELL, space="PSUM")
        )
        fin_ps = ctx.enter_context(
            tc.tile_pool(name="finps", bufs=K_FIN, space="PSUM")
        )

        # ---- persistent SBUF tensors ----
        KT = persist.tile([P, PAIRS, S], F32R)  # K^T pair-packed
        QT = persist.tile([P, PAIRS, S], F32R)  # Q^T pair-packed
        # V natural layout per (k-chunk, head): [V_h(64) | 1] in bf16
        Vp = persist.tile([P, KI, UNITS, 65], BF16)
        wq_sb = persist.tile([P, DC, 256], BF16)
        wk_sb = persist.tile([P, DC, 256], BF16)
        wv_a = persist.tile([P, 4, 256], BF16)
        wv_b = persist.tile([P, 4, 256], BF16)
        wo_sb = persist.tile([P, PAIRS, D], BF16)  # Wo^T pair-packed (K=128)
        ones_f32 = persist.tile([P, P], F32)
        ident = persist.tile([P, P], BF16)  # identity for PE transpose
        # O accumulator: [q-tile partitions, qtile, head, 64 feats + denom]
        acc = persist.tile([P, KI, UNITS, 65], F32)

        nc.vector.memset(ones_f32, 1.0)
        # identity: keep 1.0 where partition == column, else 0
        ones_bf = persist.tile([P, P], BF16)
        nc.vector.tensor_copy(ones_bf, ones_f32)
        nc.gpsimd.affine_select(
            ident,
            ones_bf,
            pattern=[[-1, P]],
            compare_op=mybir.AluOpType.is_equal,
            fill=0.0,
            base=0,
            channel_multiplier=1,
        )
        # ones columns of V' (col 64 per head); V writes only cols 0:64
        nc.vector.memset(Vp[:, :, :, 64:65], 1.0)
        # warm the ACT Exp table before the first real exp
        warm = persist.tile([P, 1], BF16)
        nc.scalar.activation(
            warm, ones_f32[:, 0:1], mybir.ActivationFunctionType.Exp, scale=1.0
        )
        # warm the PE p-state (2.4GHz needs ~3us of continuous execution):
        # junk matmuls on ones keep the PE busy while the first weight and
        # hidden DMAs land, so the real projections start at full clock
        for w in range(int(os.environ.get("K_WARM", "10"))):
            wps = fin_ps.tile([P, 64], F32, tag="fin", name="wps")
            nc.tensor.matmul(
                wps, ones_f32, ones_f32[:, 0:64], start=True, stop=True
            )
        # first-round weights split across both DMA queues so neither
        # head-blocks: K on HWDGE before the hidden quarters, Q first on
        # SWDGE; V/Wo are emitted after round 0's hidden quarters
        nc.sync.dma_start(wk_sb, wkt)
        nc.gpsimd.dma_start(wq_sb, wqt)

        # ---- cell phases ----
        pv_queue = []  # (qb, kc, pts) with exps emitted, PV pending

        def s_phase(qb, kc, p):
            """S^T + exp for one head pair of cell (qb, kc); returns pt tiles."""
            pts = []
            for k4 in range(4):
                ki = kc * 4 + k4
                stt = st_ps.tile([P, 1024], F32, tag="st")
                nc.tensor.matmul(
                    stt[:, 0:512],
                    KT[0:64, p, ts(ki, P)],
                    QT[0:64, p, ts(qb, 512)],
                    start=True,
                    stop=True,
                )
                nc.tensor.matmul(
                    stt[:, 512:1024],
                    KT[64:128, p, ts(ki, P)],
                    QT[64:128, p, ts(qb, 512)],
                    start=True,
                    stop=True,
                )
                pt = pt_pool.tile([P, 1024], BF16)
                nc.scalar.activation(
                    pt, stt, mybir.ActivationFunctionType.Exp, scale=SCALE
                )
                pts.append(pt)
            return pts

        def pv_phase(qb, kc, pts, y_on_act=False):
            """PV for cell (qb, kc): per q-tile, one psum accumulation group
            over 4 heads x 4 key chunks; accumulate into acc on DVE. On the
            final cell of a q-block, finalize each q-tile as soon as its
            accumulation lands (shortens the post-exp tail)."""
            for qt_i in range(4):
                if kc == SB - 1 and os.environ.get("K_CELLFIN", "0") == "1":
                    # final cells share the fin ring (3 bufs): their PV,
                    # transpose, and output-projection psum pipelines
                    cp = fin_ps.tile([P, UNITS, 65], F32, tag="fin", name="cpf")
                else:
                    cp = cell_ps.tile([P, UNITS, 65], F32, tag="cell")
                n = 0
                for h in range(UNITS):
                    pr, hh = divmod(h, 2)
                    for k4 in range(4):
                        nc.tensor.matmul(
                            cp[:, h, :],
                            pts[pr][k4][:, hh * 512 + qt_i * P : hh * 512 + (qt_i + 1) * P],
                            Vp[:, kc * 4 + k4, h, :],
                            start=(n == 0),
                            stop=(n == 15),
                        )
                        n += 1
                sl = acc[:, qb * 4 + qt_i, :, :]
                if kc == 0:
                    nc.vector.tensor_copy(sl, cp)
                else:
                    nc.vector.tensor_add(sl, sl, cp)
                if kc == SB - 1:
                    finalize_qt(qb * 4 + qt_i, y_on_act)

        def finalize_qt(qt, y_on_act=False):
            """Normalize q-tile qt, transpose O, output projection, DMA."""
            rc = rc_pool.tile([P, UNITS], F32)
            nc.vector.reciprocal(
                rc, acc[:, qt, :, 64:65].rearrange("p h one -> p (h one)")
            )
            otts = []
            for pr in range(PAIRS):
                otu = otu_pool.tile([P, P], BF16)
                for hh in range(2):
                    h = 2 * pr + hh
                    # all-SBUF op: runs on the otherwise-idle GPSIMD
                    nc.gpsimd.tensor_scalar_mul(
                        otu[:, hh * 64 : (hh + 1) * 64],
                        acc[:, qt, h, 0:64],
                        rc[:, h : h + 1],
                    )
                tp = fin_ps.tile([P, P], BF16, tag="fin", name="tp")
                nc.tensor.transpose(tp, otu, ident)
                ott = ott_pool.tile([P, P], BF16)
                nc.vector.tensor_copy(ott, tp)
                otts.append(ott)
            for dh in range(2):
                yps = fin_ps.tile([P, 512], F32, tag="fin", name="yps")
                for pr in range(PAIRS):
                    nc.tensor.matmul(
                        yps,
                        otts[pr],
                        wo_sb[:, pr, ds(dh * 512, 512)],
                        start=(pr == 0),
                        stop=(pr == PAIRS - 1),
                    )
                ysb = y_pool.tile([P, 512], F32)
                # the tail finalize copies y on ACT (idle once exps drain);
                # earlier ones stay on DVE to keep ACT on the exp stream
                if y_on_act:
                    nc.scalar.copy(ysb, yps)
                else:
                    nc.vector.tensor_copy(ysb, yps)
                nc.sync.dma_start(
                    y[qt * P : (qt + 1) * P, ds(dh * 512, 512)], ysb
                )

        def emit_s(qb, kc):
            pts = [s_phase(qb, kc, 0), s_phase(qb, kc, 1)]
            pv_queue.append((qb, kc, pts))

        n_final = [0]

        def drain_pv(keep, max_kc=SB - 1):
            """Emit pending PVs. max_kc gates cells whose kc-block V
            projection hasn't been emitted yet (tile deps are emission-
            ordered: a PV emitted before its V write would read stale Vp)."""
            while len(pv_queue) > keep and pv_queue[0][1] <= max_kc:
                qb, kc, pts = pv_queue.pop(0)
                if kc == SB - 1:
                    n_final[0] += 1
                pv_phase(
                    qb,
                    kc,
                    pts,
                    y_on_act=(
                        os.environ.get("K_YACT", "0") == "1"
                        and kc == SB - 1
                        and n_final[0] >= QBN
                    ),
                )

        # ---- triangular pipeline ----
        # hidden DMAs and K/Q projections run one round ahead of the cell
        # stream, so each round's cells unlock with no ACT gap; V
        # projections trail the first cell's S-phase (PV needs them ~8us
        # later).
        ht_tiles = {}

        def issue_dma(sb):
            hTa = ht_pool.tile([P, 4, 512], BF16, tag="hta", name="hTa")
            hTb = ht_pool.tile([P, 4, 512], BF16, tag="htb", name="hTb")
            if sb == 0:
                # split the cold-start load into quarters on both queues so
                # the first projection chunks begin as early as possible
                for dq in range(4):
                    nc.sync.dma_start(hTa[:, dq, :], hiddent[:, dq, ts(sb, 512)])
                    nc.gpsimd.dma_start(
                        hTb[:, dq, :], hiddent[:, 4 + dq, ts(sb, 512)]
                    )
                # V/Wo weights after the round-0 hidden (needed ~10us in)
                nc.sync.dma_start(wv_a, wvt[:, 0:4, :])
                nc.sync.dma_start(wv_b, wvt[:, 4:8, :])
                nc.sync.dma_start(wo_sb, wot)
            else:
                nc.sync.dma_start(hTa, hiddent[:, 0:4, ts(sb, 512)])
                nc.gpsimd.dma_start(hTb, hiddent[:, 4:8, ts(sb, 512)])
            ht_tiles[sb] = (hTa, hTb)

        def hts_of(sb):
            hTa, hTb = ht_tiles[sb]
            return lambda dc: (hTa if dc < 4 else hTb)[:, dc % 4, :]

        def emit_kq(sb, cell=None):
            """K/Q projections for s-block sb, pair-major. When `cell` is
            given (the first cell of round sb), each pair's S-phase is
            emitted right after that pair's two combos, so its exps keep
            ACT fed while the remaining combos run on PE."""
            hts = hts_of(sb)
            pts = []
            for p in range(PAIRS):
                for w_sb, out_t in ((wk_sb, KT), (wq_sb, QT)):
                    kps = fin_ps.tile([P, 512], F32, tag="fin", name="kps")
                    for dc in range(DC):
                        nc.tensor.matmul(
                            kps,
                            w_sb[:, dc, ts(p, P)],
                            hts(dc),
                            start=(dc == 0),
                            stop=(dc == DC - 1),
                        )
                    nc.vector.tensor_copy(out_t[:, p, ts(sb, 512)], kps)
                if cell is not None:
                    pts.append(s_phase(cell[0], cell[1], p))
            if cell is not None:
                pv_queue.append((cell[0], cell[1], pts))

        def emit_vproj(sb, st_i):
            hts = hts_of(sb)
            ki = sb * 4 + st_i
            vps = fin_ps.tile([P, 512], F32, tag="fin", name="vps")
            for dc in range(DC):
                nc.tensor.matmul(
                    vps[:, 0:256],
                    hts(dc)[:, ts(st_i, P)],
                    (wv_a if dc < 4 else wv_b)[:, dc % 4, :],
                    start=(dc == 0),
                    stop=(dc == DC - 1),
                )
            nc.vector.tensor_copy(
                Vp[:, ki, :, 0:64],
                vps[:, 0:256].rearrange("p (h f) -> p h f", h=UNITS),
            )

        issue_dma(0)
        # round 0's K/Q fused with cell (0,0)'s S-phases
        emit_kq(0, cell=(0, 0))
        for sb in range(SB):
            if sb + 1 < SB:
                issue_dma(sb + 1)
            # newly-ready cells: earlier q-blocks against this round's keys,
            # plus this q-block against all keys so far. Cell (0, sb) was
            # already emitted fused into emit_kq(sb).
            new_cells = [(qb, sb) for qb in range(1, sb)]
            new_cells += [(sb, kc) for kc in range(sb + 1)]
            if sb == 0:
                new_cells = []  # (0, 0) already emitted fused in emit_kq(0)
            for st_i in range(4):
                emit_vproj(sb, st_i)
            drain_pv(keep=1)
            for cell in new_cells:
                emit_s(*cell)
                drain_pv(keep=1)
            # next round's K/Q (+ its first cell) at the round's lowest
            # priority: PE falls back to them when the cell stream stalls
            if sb + 1 < SB:
                emit_kq(sb + 1, cell=(0, sb + 1))
        drain_pv(keep=0)
    nc.compile()
    return nc


_NC = None


def get_nc():
    global _NC
    if _NC is None:
        _NC = build_nc()
    return _NC


def shard_inputs(hidden_states, Wq, Wk, Wv, Wo):
    """Per-core input maps. Core c: batch c//4, heads 4*(c%4) .. 4*(c%4)+3."""
    import ml_dtypes

    hidden_states = np.asarray(hidden_states, np.float32)
    Wq, Wk, Wv, Wo = (np.asarray(w, np.float32) for w in (Wq, Wk, Wv, Wo))
    in_maps = []
    for c in range(N_CORES):
        b = c // 4
        f0 = (c % 4) * 4 * DIM_HEAD  # first feature row/col of this core's heads
        rows = slice(f0, f0 + UNITS * DIM_HEAD)

        def proj_layout(w):
            # W[rows].T is [D, 256]; on-chip layout is [128, DC, 256] bf16
            return np.ascontiguousarray(
                w[rows, :].T.reshape(DC, P, 256).transpose(1, 0, 2)
            ).astype(ml_dtypes.bfloat16)

        # Wo[:, rows].T is [256, D]; on-chip layout is [128, PAIRS, D] bf16
        wot = np.ascontiguousarray(
            Wo[:, rows].T.reshape(PAIRS, P, D).transpose(1, 0, 2)
        ).astype(ml_dtypes.bfloat16)
        in_maps.append(
            {
                "hiddent": np.ascontiguousarray(
                    hidden_states[b].T.reshape(DC, P, S).transpose(1, 0, 2)
                ).astype(ml_dtypes.bfloat16),
                "wqt": proj_layout(Wq),
                "wkt": proj_layout(Wk),
                "wvt": proj_layout(Wv),
                "wot": wot,
            }
        )
    return in_maps


def unshard_outputs(results, bo):
    out = np.zeros((B, S, D), np.float32)
    for c, res in enumerate(results):
        out[c // 4] += res["y"]
    out += np.asarray(bo, np.float32)[None, None, :]
    return out


def kernel(hidden_states, Wq, Wk, Wv, Wo, bo, _trace=False):
    from concourse.bass_utils import run_bass_kernel_spmd

    nc = get_nc()
    in_maps = shard_inputs(hidden_states, Wq, Wk, Wv, Wo)
    res = run_bass_kernel_spmd(nc, in_maps, list(range(N_CORES)), trace=_trace)
    out = unshard_outputs(res.results, bo)
    if _trace:
        return out, res
    return out


# revision 36
# speedup vs baseline: 1.0118x; 1.0039x over previous
# CrossAttention (B=2, S=2048, D=1024, H=16, dh=64) on 8 trn2 NeuronCores.
#
# Sharding: 32 (batch, head) units, 4 consecutive units per core (cores 0-3
# work on batch 0, cores 4-7 on batch 1). Each core receives its batch's
# hidden states pre-permuted to the on-chip [128, D/128, S] transposed
# layout, per-head slices of Wq/Wk/Wv/Wo (also pre-permuted); it returns a
# partial output y [2048, 1024] (its heads' contribution to the output
# projection). The host sums the four partials per batch and adds bo.
#
# Device algorithm (per core, 4 heads = 2 pairs). The cost model prices a
# matmul at (moving free size) x (cycles/row): fp32r is 1 cyc/row only at
# N>=256, bf16 is 1 cyc/row at any N, and M/K are free. The kernel exploits
# this by keeping the big-N matmuls (projections, scores) in fp32r and
# running PV "swapped" with tiny-N bf16 moving operands:
#   - projections: Q^T/K^T pair-packed [128, s] fp32r (N=512); V in natural
#     [s, feat] layout converted to bf16 with a fused ones column per head.
#   - scores: S^T = K^T-chunk^T Q^T per pair (K=64, M=128 keys, N=512).
#   - P^T = exp(S^T/8) on ACT, written as bf16 (PV stationary).
#   - PV swapped: stationary = P^T tile [128 keys, 128 q], moving =
#     [V_h | 1] bf16 [128 keys, 65] -> psum [128 q, 65] per head; one psum
#     accumulation group per (cell, qtile) over 4 key chunks x 4 heads;
#     column 64 accumulates the softmax denominator. Half the PE cycles of
#     the classic V'^T P^T orientation, and O lands in [q, feat] layout.
#   - normalize: DVE reciprocal of psum col 64, per-partition
#     tensor_scalar_mul -> bf16 O tiles; PE bf16 transpose (identity
#     permutation) -> O^T [feat, q]; output projection with K=128 (head
#     pairs packed) and fp32r moving Wo^T (N=512).
# Triangular software pipeline over 512-wide s-block rounds keeps the ACT
# engine (exp is ~equal to total PE work) fed from round 0.
import os
import sys

import numpy as np

try:
    import concourse.bass as bass
except ImportError:  # harness runs from a fresh dir; repo is on the default path
    sys.path.insert(0, "/opt/trn_rl_repo")
    import concourse.bass as bass

import concourse.bacc as bacc
import concourse.mybir as mybir
import concourse.tile as tile
from concourse.bass import ts, ds
from contextlib import ExitStack

B, S, D = 2, 2048, 1024
HEADS, DIM_HEAD = 16, 64
SCALE = DIM_HEAD**-0.5
N_CORES = 8
UNITS = 4  # heads per core
PAIRS = 2  # head pairs per core
P = 128
SB = S // 512  # 4 s-block rounds (key blocks)
QBN = S // 512  # 4 q-blocks
DC = D // P  # 8 contraction chunks for projections
KI = S // P  # 16 key chunks of 128
F32 = mybir.dt.float32
F32R = mybir.dt.float32r
BF16 = mybir.dt.bfloat16


def build_nc():
    nc = bacc.Bacc("TRN2", target_bir_lowering=False, debug=False)

    hiddent = nc.dram_tensor("hiddent", [P, DC, S], BF16, kind="ExternalInput").ap()
    wqt = nc.dram_tensor("wqt", [P, DC, 256], BF16, kind="ExternalInput").ap()
    wkt = nc.dram_tensor("wkt", [P, DC, 256], BF16, kind="ExternalInput").ap()
    wvt = nc.dram_tensor("wvt", [P, DC, 256], BF16, kind="ExternalInput").ap()
    # Wo^T pair-packed: wot[f, pair, d] = Wo[d, f0 + pair*128 + f]; bf16 so
    # the O-projection (bf16 O^T stationary) has matching input dtypes
    wot = nc.dram_tensor("wot", [P, PAIRS, D], BF16, kind="ExternalInput").ap()
    y = nc.dram_tensor("y", [S, D], F32, kind="ExternalOutput").ap()

    K_PT = int(os.environ.get("K_PT", "28"))
    K_ST = int(os.environ.get("K_ST", "2"))
    K_CELL = int(os.environ.get("K_CELL", "1"))
    K_FIN = int(os.environ.get("K_FIN", "3"))
    K_DEFER = int(os.environ.get("K_DEFER", "0"))
    K_LASTFIRST = os.environ.get("K_LASTFIRST", "0") == "1"

    with tile.TileContext(nc) as tc, ExitStack() as ctx:
        persist = ctx.enter_context(tc.tile_pool(name="persist", bufs=1))
        pt_pool = ctx.enter_context(tc.tile_pool(name="pt", bufs=K_PT))
        otu_pool = ctx.enter_context(tc.tile_pool(name="otu", bufs=6))
        ott_pool = ctx.enter_context(tc.tile_pool(name="ott", bufs=6))
        rc_pool = ctx.enter_context(tc.tile_pool(name="rc", bufs=4))
        y_pool = ctx.enter_context(tc.tile_pool(name="ysb", bufs=4))
        ht_pool = ctx.enter_context(tc.tile_pool(name="htp", bufs=2))
        # PSUM (8 banks): S^T tiles 2x[128,1024] (4 banks), PV accumulators
        # 2x[128,4x65] (2 banks), projection/transpose/output transients
        # 2x[128,512] (2 banks).
        st_ps = ctx.enter_context(
            tc.tile_pool(name="stps", bufs=K_ST, space="PSUM")
        )
        cell_ps = ctx.enter_context(
            tc.tile_pool(name="cellps", bufs=K_CELL, space="PSUM")
        )
        fin_ps = ctx.enter_context(
            tc.tile_pool(name="finps", bufs=K_FIN, space="PSUM")
        )

        # ---- persistent SBUF tensors ----
        KT = persist.tile([P, PAIRS, S], F32R)  # K^T pair-packed
        QT = persist.tile([P, PAIRS, S], F32R)  # Q^T pair-packed
        # V natural layout per (k-chunk, head): [V_h(64) | 1] in bf16
        Vp = persist.tile([P, KI, UNITS, 65], BF16)
        wq_sb = persist.tile([P, DC, 256], BF16)
        wk_sb = persist.tile([P, DC, 256], BF16)
        wv_a = persist.tile([P, 4, 256], BF16)
        wv_b = persist.tile([P, 4, 256], BF16)
        wo_sb = persist.tile([P, PAIRS, D], BF16)  # Wo^T pair-packed (K=128)
        ones_f32 = persist.tile([P, P], F32)
        ident = persist.tile([P, P], BF16)  # identity for PE transpose
        # O accumulator: [q-tile partitions, qtile, head, 64 feats + denom]
        acc = persist.tile([P, KI, UNITS, 65], F32)

        nc.vector.memset(ones_f32, 1.0)
        # identity: keep 1.0 where partition == column, else 0
        ones_bf = persist.tile([P, P], BF16)
        nc.vector.tensor_copy(ones_bf, ones_f32)
        nc.gpsimd.affine_select(
            ident,
            ones_bf,
            pattern=[[-1, P]],
            compare_op=mybir.AluOpType.is_equal,
            fill=0.0,
            base=0,
            channel_multiplier=1,
        )
        # ones columns of V' (col 64 per head); V writes only cols 0:64
        nc.vector.memset(Vp[:, :, :, 64:65], 1.0)
        # warm the ACT Exp table before the first real exp
        warm = persist.tile([P, 1], BF16)
        nc.scalar.activation(
            warm, ones_f32[:, 0:1], mybir.ActivationFunctionType.Exp, scale=1.0
        )
        # warm the PE p-state (2.4GHz needs ~3us of continuous execution):
        # junk matmuls on ones keep the PE busy while the first weight and
        # hidden DMAs land, so the real projections start at full clock
        for w in range(int(os.environ.get("K_WARM", "10"))):
            wps = fin_ps.tile([P, 64], F32, tag="fin", name="wps")
            nc.tensor.matmul(
                wps, ones_f32, ones_f32[:, 0:64], start=True, stop=True
            )
        # first-round weights split across both DMA queues so neither
        # head-blocks: K on HWDGE before the hidden quarters, Q first on
        # SWDGE; V/Wo are emitted after round 0's hidden quarters
        nc.sync.dma_start(wk_sb, wkt)
        nc.gpsimd.dma_start(wq_sb, wqt)

        # ---- cell phases ----
        pv_queue = []  # (qb, kc, pts) with exps emitted, PV pending

        def s_phase(qb, kc, p):
            """S^T + exp for one head pair of cell (qb, kc); returns pt tiles."""
            pts = []
            for k4 in range(4):
                ki = kc * 4 + k4
                stt = st_ps.tile([P, 1024], F32, tag="st")
                nc.tensor.matmul(
                    stt[:, 0:512],
                    KT[0:64, p, ts(ki, P)],
                    QT[0:64, p, ts(qb, 512)],
                    start=True,
                    stop=True,
                )
                nc.tensor.matmul(
                    stt[:, 512:1024],
                    KT[64:128, p, ts(ki, P)],
                    QT[64:128, p, ts(qb, 512)],
                    start=True,
                    stop=True,
                )
                pt = pt_pool.tile([P, 1024], BF16)
                nc.scalar.activation(
                    pt, stt, mybir.ActivationFunctionType.Exp, scale=SCALE
                )
                pts.append(pt)
            return pts

        def pv_phase(qb, kc, pts, y_on_act=False):
            """PV for cell (qb, kc): per q-tile, one psum accumulation group
            over 4 heads x 4 key chunks; accumulate into acc on DVE. On the
            final cell of a q-block, finalize each q-tile as soon as its
            accumulation lands (shortens the post-exp tail)."""
            for qt_i in range(4):
                if kc == SB - 1 and os.environ.get("K_CELLF", "1") == "1":
                    # final cells alternate two psum tags so the four
                    # trailing qt chains pipeline two-deep
                    cp = cell_ps.tile(
                        [P, UNITS, 65],
                        F32,
                        tag="cell" if qt_i % 2 == 0 else "cellf",
                        bufs=1,
                        name="cp",
                    )
                else:
                    cp = cell_ps.tile([P, UNITS, 65], F32, tag="cell", bufs=1, name="cp")
                n = 0
                for h in range(UNITS):
                    pr, hh = divmod(h, 2)
                    for k4 in range(4):
                        nc.tensor.matmul(
                            cp[:, h, :],
                            pts[pr][k4][:, hh * 512 + qt_i * P : hh * 512 + (qt_i + 1) * P],
                            Vp[:, kc * 4 + k4, h, :],
                            start=(n == 0),
                            stop=(n == 15),
                        )
                        n += 1
                sl = acc[:, qb * 4 + qt_i, :, :]
                if kc == 0:
                    nc.vector.tensor_copy(sl, cp)
                else:
                    nc.vector.tensor_add(sl, sl, cp)
                if kc == SB - 1:
                    finalize_qt(qb * 4 + qt_i, y_on_act)

        def finalize_qt(qt, y_on_act=False):
            """Normalize q-tile qt, transpose O, output projection, DMA."""
            rc = rc_pool.tile([P, UNITS], F32)
            nc.vector.reciprocal(
                rc, acc[:, qt, :, 64:65].rearrange("p h one -> p (h one)")
            )
            otts = []
            for pr in range(PAIRS):
                otu = otu_pool.tile([P, P], BF16)
                for hh in range(2):
                    h = 2 * pr + hh
                    # all-SBUF op: runs on the otherwise-idle GPSIMD
                    nc.gpsimd.tensor_scalar_mul(
                        otu[:, hh * 64 : (hh + 1) * 64],
                        acc[:, qt, h, 0:64],
                        rc[:, h : h + 1],
                    )
                tp = fin_ps.tile([P, P], BF16, tag="fin", name="tp")
                nc.tensor.transpose(tp, otu, ident)
                ott = ott_pool.tile([P, P], BF16)
                nc.vector.tensor_copy(ott, tp)
                otts.append(ott)
            for dh in range(2):
                yps = fin_ps.tile([P, 512], F32, tag="fin", name="yps")
                for pr in range(PAIRS):
                    nc.tensor.matmul(
                        yps,
                        otts[pr],
                        wo_sb[:, pr, ds(dh * 512, 512)],
                        start=(pr == 0),
                        stop=(pr == PAIRS - 1),
                    )
                ysb = y_pool.tile([P, 512], F32)
                # the tail finalize copies y on ACT (idle once exps drain);
                # earlier ones stay on DVE to keep ACT on the exp stream
                if y_on_act:
                    nc.scalar.copy(ysb, yps)
                else:
                    nc.vector.tensor_copy(ysb, yps)
                nc.sync.dma_start(
                    y[qt * P : (qt + 1) * P, ds(dh * 512, 512)], ysb
                )

        def emit_s(qb, kc):
            pts = [s_phase(qb, kc, 0), s_phase(qb, kc, 1)]
            pv_queue.append((qb, kc, pts))

        n_final = [0]

        def drain_pv(keep, max_kc=SB - 1):
            """Emit pending PVs. max_kc gates cells whose kc-block V
            projection hasn't been emitted yet (tile deps are emission-
            ordered: a PV emitted before its V write would read stale Vp)."""
            while len(pv_queue) > keep and pv_queue[0][1] <= max_kc:
                qb, kc, pts = pv_queue.pop(0)
                if kc == SB - 1:
                    n_final[0] += 1
                pv_phase(
                    qb,
                    kc,
                    pts,
                    y_on_act=(
                        os.environ.get("K_YACT", "0") == "1"
                        and kc == SB - 1
                        and n_final[0] >= QBN
                    ),
                )

        # ---- triangular pipeline ----
        # hidden DMAs and K/Q projections run one round ahead of the cell
        # stream, so each round's cells unlock with no ACT gap; V
        # projections trail the first cell's S-phase (PV needs them ~8us
        # later).
        ht_tiles = {}

        def issue_dma(sb):
            hTa = ht_pool.tile([P, 4, 512], BF16, tag="hta", name="hTa")
            hTb = ht_pool.tile([P, 4, 512], BF16, tag="htb", name="hTb")
            if sb == 0:
                # split the cold-start load into quarters on both queues so
                # the first projection chunks begin as early as possible
                for dq in range(4):
                    nc.sync.dma_start(hTa[:, dq, :], hiddent[:, dq, ts(sb, 512)])
                    nc.gpsimd.dma_start(
                        hTb[:, dq, :], hiddent[:, 4 + dq, ts(sb, 512)]
                    )
                # V/Wo weights after the round-0 hidden (needed ~10us in)
                nc.sync.dma_start(wv_a, wvt[:, 0:4, :])
                nc.sync.dma_start(wv_b, wvt[:, 4:8, :])
                nc.sync.dma_start(wo_sb, wot)
            else:
                nc.sync.dma_start(hTa, hiddent[:, 0:4, ts(sb, 512)])
                nc.gpsimd.dma_start(hTb, hiddent[:, 4:8, ts(sb, 512)])
            ht_tiles[sb] = (hTa, hTb)

        def hts_of(sb):
            hTa, hTb = ht_tiles[sb]
            return lambda dc: (hTa if dc < 4 else hTb)[:, dc % 4, :]

        def emit_kq(sb, cell=None):
            """K/Q projections for s-block sb, pair-major. When `cell` is
            given (the first cell of round sb), each pair's S-phase is
            emitted right after that pair's two combos, so its exps keep
            ACT fed while the remaining combos run on PE."""
            hts = hts_of(sb)
            pts = []
            for p in range(PAIRS):
                for w_sb, out_t in ((wk_sb, KT), (wq_sb, QT)):
                    kps = fin_ps.tile([P, 512], F32, tag="fin", name="kps")
                    for dc in range(DC):
                        nc.tensor.matmul(
                            kps,
                            w_sb[:, dc, ts(p, P)],
                            hts(dc),
                            start=(dc == 0),
                            stop=(dc == DC - 1),
                        )
                    nc.vector.tensor_copy(out_t[:, p, ts(sb, 512)], kps)
                if cell is not None:
                    pts.append(s_phase(cell[0], cell[1], p))
            if cell is not None:
                pv_queue.append((cell[0], cell[1], pts))

        def emit_vproj(sb, st_i):
            hts = hts_of(sb)
            ki = sb * 4 + st_i
            vps = fin_ps.tile([P, 512], F32, tag="fin", name="vps")
            for dc in range(DC):
                nc.tensor.matmul(
                    vps[:, 0:256],
                    hts(dc)[:, ts(st_i, P)],
                    (wv_a if dc < 4 else wv_b)[:, dc % 4, :],
                    start=(dc == 0),
                    stop=(dc == DC - 1),
                )
            nc.vector.tensor_copy(
                Vp[:, ki, :, 0:64],
                vps[:, 0:256].rearrange("p (h f) -> p h f", h=UNITS),
            )

        issue_dma(0)
        # round 0's K/Q fused with cell (0,0)'s S-phases
        emit_kq(0, cell=(0, 0))
        for sb in range(SB):
            if sb + 1 < SB:
                issue_dma(sb + 1)
            # newly-ready cells: earlier q-blocks against this round's keys,
            # plus this q-block against all keys so far. Cell (0, sb) was
            # already emitted fused into emit_kq(sb).
            new_cells = [(qb, sb) for qb in range(1, sb)]
            new_cells += [(sb, kc) for kc in range(sb + 1)]
            if sb == 0:
                new_cells = []  # (0, 0) already emitted fused in emit_kq(0)
            for st_i in range(4):
                emit_vproj(sb, st_i)
            drain_pv(keep=1)
            for cell in new_cells:
                emit_s(*cell)
                drain_pv(keep=1)
            # next round's K/Q (+ its first cell) at the round's lowest
            # priority: PE falls back to them when the cell stream stalls
            if sb + 1 < SB:
                emit_kq(sb + 1, cell=(0, sb + 1))
        drain_pv(keep=0)
    nc.compile()
    return nc


_NC = None


def get_nc():
    global _NC
    if _NC is None:
        _NC = build_nc()
    return _NC


def shard_inputs(hidden_states, Wq, Wk, Wv, Wo):
    """Per-core input maps. Core c: batch c//4, heads 4*(c%4) .. 4*(c%4)+3."""
    import ml_dtypes

    hidden_states = np.asarray(hidden_states, np.float32)
    Wq, Wk, Wv, Wo = (np.asarray(w, np.float32) for w in (Wq, Wk, Wv, Wo))
    in_maps = []
    for c in range(N_CORES):
        b = c // 4
        f0 = (c % 4) * 4 * DIM_HEAD  # first feature row/col of this core's heads
        rows = slice(f0, f0 + UNITS * DIM_HEAD)

        def proj_layout(w):
            # W[rows].T is [D, 256]; on-chip layout is [128, DC, 256] bf16
            return np.ascontiguousarray(
                w[rows, :].T.reshape(DC, P, 256).transpose(1, 0, 2)
            ).astype(ml_dtypes.bfloat16)

        # Wo[:, rows].T is [256, D]; on-chip layout is [128, PAIRS, D] bf16
        wot = np.ascontiguousarray(
            Wo[:, rows].T.reshape(PAIRS, P, D).transpose(1, 0, 2)
        ).astype(ml_dtypes.bfloat16)
        in_maps.append(
            {
                "hiddent": np.ascontiguousarray(
                    hidden_states[b].T.reshape(DC, P, S).transpose(1, 0, 2)
                ).astype(ml_dtypes.bfloat16),
                "wqt": proj_layout(Wq),
                "wkt": proj_layout(Wk),
                "wvt": proj_layout(Wv),
                "wot": wot,
            }
        )
    return in_maps


def unshard_outputs(results, bo):
    out = np.zeros((B, S, D), np.float32)
    for c, res in enumerate(results):
        out[c // 4] += res["y"]
    out += np.asarray(bo, np.float32)[None, None, :]
    return out


def kernel(hidden_states, Wq, Wk, Wv, Wo, bo, _trace=False):
    from concourse.bass_utils import run_bass_kernel_spmd

    nc = get_nc()
    in_maps = shard_inputs(hidden_states, Wq, Wk, Wv, Wo)
    res = run_bass_kernel_spmd(nc, in_maps, list(range(N_CORES)), trace=_trace)
    out = unshard_outputs(res.results, bo)
    if _trace:
        return out, res
    return out


# revision 37
# speedup vs baseline: 1.0204x; 1.0085x over previous
# CrossAttention (B=2, S=2048, D=1024, H=16, dh=64) on 8 trn2 NeuronCores.
#
# Sharding: 32 (batch, head) units, 4 consecutive units per core (cores 0-3
# work on batch 0, cores 4-7 on batch 1). Each core receives its batch's
# hidden states pre-permuted to the on-chip [128, D/128, S] transposed
# layout, per-head slices of Wq/Wk/Wv/Wo (also pre-permuted); it returns a
# partial output y [2048, 1024] (its heads' contribution to the output
# projection). The host sums the four partials per batch and adds bo.
#
# Device algorithm (per core, 4 heads = 2 pairs). The cost model prices a
# matmul at (moving free size) x (cycles/row): fp32r is 1 cyc/row only at
# N>=256, bf16 is 1 cyc/row at any N, and M/K are free. The kernel exploits
# this by keeping the big-N matmuls (projections, scores) in fp32r and
# running PV "swapped" with tiny-N bf16 moving operands:
#   - projections: Q^T/K^T pair-packed [128, s] fp32r (N=512); V in natural
#     [s, feat] layout converted to bf16 with a fused ones column per head.
#   - scores: S^T = K^T-chunk^T Q^T per pair (K=64, M=128 keys, N=512).
#   - P^T = exp(S^T/8) on ACT, written as bf16 (PV stationary).
#   - PV swapped: stationary = P^T tile [128 keys, 128 q], moving =
#     [V_h | 1] bf16 [128 keys, 65] -> psum [128 q, 65] per head; one psum
#     accumulation group per (cell, qtile) over 4 key chunks x 4 heads;
#     column 64 accumulates the softmax denominator. Half the PE cycles of
#     the classic V'^T P^T orientation, and O lands in [q, feat] layout.
#   - normalize: DVE reciprocal of psum col 64, per-partition
#     tensor_scalar_mul -> bf16 O tiles; PE bf16 transpose (identity
#     permutation) -> O^T [feat, q]; output projection with K=128 (head
#     pairs packed) and fp32r moving Wo^T (N=512).
# Triangular software pipeline over 512-wide s-block rounds keeps the ACT
# engine (exp is ~equal to total PE work) fed from round 0.
import os
import sys

import numpy as np

try:
    import concourse.bass as bass
except ImportError:  # harness runs from a fresh dir; repo is on the default path
    sys.path.insert(0, "/opt/trn_rl_repo")
    import concourse.bass as bass

import concourse.bacc as bacc
import concourse.mybir as mybir
import concourse.tile as tile
from concourse.bass import ts, ds
from contextlib import ExitStack

B, S, D = 2, 2048, 1024
HEADS, DIM_HEAD = 16, 64
SCALE = DIM_HEAD**-0.5
N_CORES = 8
UNITS = 4  # heads per core
PAIRS = 2  # head pairs per core
P = 128
SB = S // 512  # 4 s-block rounds (key blocks)
QBN = S // 512  # 4 q-blocks
DC = D // P  # 8 contraction chunks for projections
KI = S // P  # 16 key chunks of 128
F32 = mybir.dt.float32
F32R = mybir.dt.float32r
BF16 = mybir.dt.bfloat16


def build_nc():
    nc = bacc.Bacc("TRN2", target_bir_lowering=False, debug=False)

    hiddent = nc.dram_tensor("hiddent", [P, DC, S], BF16, kind="ExternalInput").ap()
    wqt = nc.dram_tensor("wqt", [P, DC, 256], BF16, kind="ExternalInput").ap()
    wkt = nc.dram_tensor("wkt", [P, DC, 256], BF16, kind="ExternalInput").ap()
    wvt = nc.dram_tensor("wvt", [P, DC, 256], BF16, kind="ExternalInput").ap()
    # Wo^T pair-packed: wot[f, pair, d] = Wo[d, f0 + pair*128 + f]; bf16 so
    # the O-projection (bf16 O^T stationary) has matching input dtypes
    wot = nc.dram_tensor("wot", [P, PAIRS, D], BF16, kind="ExternalInput").ap()
    y = nc.dram_tensor("y", [S, D], F32, kind="ExternalOutput").ap()

    K_PT = int(os.environ.get("K_PT", "28"))
    K_ST = int(os.environ.get("K_ST", "2"))
    K_CELL = int(os.environ.get("K_CELL", "1"))
    K_FIN = int(os.environ.get("K_FIN", "2"))
    K_DEFER = int(os.environ.get("K_DEFER", "0"))
    K_LASTFIRST = os.environ.get("K_LASTFIRST", "0") == "1"

    with tile.TileContext(nc) as tc, ExitStack() as ctx:
        persist = ctx.enter_context(tc.tile_pool(name="persist", bufs=1))
        pt_pool = ctx.enter_context(tc.tile_pool(name="pt", bufs=K_PT))
        otu_pool = ctx.enter_context(tc.tile_pool(name="otu", bufs=6))
        ott_pool = ctx.enter_context(tc.tile_pool(name="ott", bufs=6))
        rc_pool = ctx.enter_context(tc.tile_pool(name="rc", bufs=4))
        y_pool = ctx.enter_context(tc.tile_pool(name="ysb", bufs=4))
        ht_pool = ctx.enter_context(tc.tile_pool(name="htp", bufs=2))
        # PSUM (8 banks): S^T tiles 2x[128,1024] (4 banks), PV accumulators
        # 2x[128,4x65] (2 banks), projection/transpose/output transients
        # 2x[128,512] (2 banks).
        st_ps = ctx.enter_context(
            tc.tile_pool(name="stps", bufs=K_ST, space="PSUM")
        )
        cell_ps = ctx.enter_context(
            tc.tile_pool(name="cellps", bufs=K_CELL, space="PSUM")
        )
        fin_ps = ctx.enter_context(
            tc.tile_pool(name="finps", bufs=K_FIN, space="PSUM")
        )

        # ---- persistent SBUF tensors ----
        KT = persist.tile([P, PAIRS, S], F32R)  # K^T pair-packed
        QT = persist.tile([P, PAIRS, S], F32R)  # Q^T pair-packed
        # V natural layout per (k-chunk, head): [V_h(64) | 1] in bf16
        Vp = persist.tile([P, KI, UNITS, 65], BF16)
        wq_sb = persist.tile([P, DC, 256], BF16)
        wk_sb = persist.tile([P, DC, 256], BF16)
        wv_a = persist.tile([P, 4, 256], BF16)
        wv_b = persist.tile([P, 4, 256], BF16)
        wo_sb = persist.tile([P, PAIRS, D], BF16)  # Wo^T pair-packed (K=128)
        ones_f32 = persist.tile([P, P], F32)
        ident = persist.tile([P, P], BF16)  # identity for PE transpose
        # O accumulator: [q-tile partitions, qtile, head, 64 feats + denom]
        acc = persist.tile([P, KI, UNITS, 65], F32)

        nc.vector.memset(ones_f32, 1.0)
        # identity: keep 1.0 where partition == column, else 0
        ones_bf = persist.tile([P, P], BF16)
        nc.vector.tensor_copy(ones_bf, ones_f32)
        nc.gpsimd.affine_select(
            ident,
            ones_bf,
            pattern=[[-1, P]],
            compare_op=mybir.AluOpType.is_equal,
            fill=0.0,
            base=0,
            channel_multiplier=1,
        )
        # ones columns of V' (col 64 per head); V writes only cols 0:64
        nc.vector.memset(Vp[:, :, :, 64:65], 1.0)
        # warm the ACT Exp table before the first real exp
        warm = persist.tile([P, 1], BF16)
        nc.scalar.activation(
            warm, ones_f32[:, 0:1], mybir.ActivationFunctionType.Exp, scale=1.0
        )
        # warm the PE p-state (2.4GHz needs ~3us of continuous execution):
        # junk matmuls on ones keep the PE busy while the first weight and
        # hidden DMAs land, so the real projections start at full clock
        for w in range(int(os.environ.get("K_WARM", "10"))):
            wps = fin_ps.tile([P, 64], F32, tag="fin", name="wps")
            nc.tensor.matmul(
                wps, ones_f32, ones_f32[:, 0:64], start=True, stop=True
            )
        # first-round weights split across both DMA queues so neither
        # head-blocks: K on HWDGE before the hidden quarters, Q first on
        # SWDGE; V/Wo are emitted after round 0's hidden quarters
        nc.sync.dma_start(wk_sb, wkt)
        nc.gpsimd.dma_start(wq_sb, wqt)

        # ---- cell phases ----
        pv_queue = []  # (qb, kc, pts) with exps emitted, PV pending

        def s_phase(qb, kc, p):
            """S^T + exp for one head pair of cell (qb, kc); returns pt tiles."""
            pts = []
            for k4 in range(4):
                ki = kc * 4 + k4
                stt = st_ps.tile([P, 1024], F32, tag="st")
                nc.tensor.matmul(
                    stt[:, 0:512],
                    KT[0:64, p, ts(ki, P)],
                    QT[0:64, p, ts(qb, 512)],
                    start=True,
                    stop=True,
                )
                nc.tensor.matmul(
                    stt[:, 512:1024],
                    KT[64:128, p, ts(ki, P)],
                    QT[64:128, p, ts(qb, 512)],
                    start=True,
                    stop=True,
                )
                pt = pt_pool.tile([P, 1024], BF16)
                nc.scalar.activation(
                    pt, stt, mybir.ActivationFunctionType.Exp, scale=SCALE
                )
                pts.append(pt)
            return pts

        def pv_phase(qb, kc, pts, y_on_act=False):
            """PV for cell (qb, kc): per q-tile, one psum accumulation group
            over 4 heads x 4 key chunks; accumulate into acc on DVE. On the
            final cell of a q-block, finalize each q-tile as soon as its
            accumulation lands (shortens the post-exp tail)."""
            for qt_i in range(4):
                if os.environ.get("K_CELLFALL", "0") == "1" or (
                    kc == SB - 1 and os.environ.get("K_CELLF", "1") == "1"
                ):
                    # final cells alternate two psum tags so the four
                    # trailing qt chains pipeline two-deep
                    cp = cell_ps.tile(
                        [P, UNITS, 65],
                        F32,
                        tag="cell" if qt_i % 2 == 0 else "cellf",
                        bufs=1,
                        name="cp",
                    )
                else:
                    cp = cell_ps.tile([P, UNITS, 65], F32, tag="cell", bufs=1, name="cp")
                n = 0
                for h in range(UNITS):
                    pr, hh = divmod(h, 2)
                    for k4 in range(4):
                        nc.tensor.matmul(
                            cp[:, h, :],
                            pts[pr][k4][:, hh * 512 + qt_i * P : hh * 512 + (qt_i + 1) * P],
                            Vp[:, kc * 4 + k4, h, :],
                            start=(n == 0),
                            stop=(n == 15),
                        )
                        n += 1
                sl = acc[:, qb * 4 + qt_i, :, :]
                if kc == 0:
                    nc.vector.tensor_copy(sl, cp)
                else:
                    nc.vector.tensor_add(sl, sl, cp)
                if kc == SB - 1:
                    finalize_qt(qb * 4 + qt_i, y_on_act)

        def finalize_qt(qt, y_on_act=False):
            """Normalize q-tile qt, transpose O, output projection, DMA."""
            rc = rc_pool.tile([P, UNITS], F32)
            nc.vector.reciprocal(
                rc, acc[:, qt, :, 64:65].rearrange("p h one -> p (h one)")
            )
            otts = []
            for pr in range(PAIRS):
                otu = otu_pool.tile([P, P], BF16)
                for hh in range(2):
                    h = 2 * pr + hh
                    # all-SBUF op: runs on the otherwise-idle GPSIMD
                    nc.gpsimd.tensor_scalar_mul(
                        otu[:, hh * 64 : (hh + 1) * 64],
                        acc[:, qt, h, 0:64],
                        rc[:, h : h + 1],
                    )
                tp = fin_ps.tile([P, P], BF16, tag="fin", name="tp")
                nc.tensor.transpose(tp, otu, ident)
                ott = ott_pool.tile([P, P], BF16)
                nc.vector.tensor_copy(ott, tp)
                otts.append(ott)
            for dh in range(2):
                yps = fin_ps.tile([P, 512], F32, tag="fin", name="yps")
                for pr in range(PAIRS):
                    nc.tensor.matmul(
                        yps,
                        otts[pr],
                        wo_sb[:, pr, ds(dh * 512, 512)],
                        start=(pr == 0),
                        stop=(pr == PAIRS - 1),
                    )
                ysb = y_pool.tile([P, 512], F32)
                # the tail finalize copies y on ACT (idle once exps drain);
                # earlier ones stay on DVE to keep ACT on the exp stream
                if y_on_act:
                    nc.scalar.copy(ysb, yps)
                else:
                    nc.vector.tensor_copy(ysb, yps)
                nc.sync.dma_start(
                    y[qt * P : (qt + 1) * P, ds(dh * 512, 512)], ysb
                )

        def emit_s(qb, kc):
            pts = [s_phase(qb, kc, 0), s_phase(qb, kc, 1)]
            pv_queue.append((qb, kc, pts))

        n_final = [0]

        def drain_pv(keep, max_kc=SB - 1):
            """Emit pending PVs. max_kc gates cells whose kc-block V
            projection hasn't been emitted yet (tile deps are emission-
            ordered: a PV emitted before its V write would read stale Vp)."""
            while len(pv_queue) > keep and pv_queue[0][1] <= max_kc:
                qb, kc, pts = pv_queue.pop(0)
                if kc == SB - 1:
                    n_final[0] += 1
                pv_phase(
                    qb,
                    kc,
                    pts,
                    y_on_act=(
                        os.environ.get("K_YACT", "0") == "1"
                        and kc == SB - 1
                        and n_final[0] >= QBN
                    ),
                )

        # ---- triangular pipeline ----
        # hidden DMAs and K/Q projections run one round ahead of the cell
        # stream, so each round's cells unlock with no ACT gap; V
        # projections trail the first cell's S-phase (PV needs them ~8us
        # later).
        ht_tiles = {}

        def issue_dma(sb):
            hTa = ht_pool.tile([P, 4, 512], BF16, tag="hta", name="hTa")
            hTb = ht_pool.tile([P, 4, 512], BF16, tag="htb", name="hTb")
            if sb == 0:
                # split the cold-start load into quarters on both queues so
                # the first projection chunks begin as early as possible
                for dq in range(4):
                    nc.sync.dma_start(hTa[:, dq, :], hiddent[:, dq, ts(sb, 512)])
                    nc.gpsimd.dma_start(
                        hTb[:, dq, :], hiddent[:, 4 + dq, ts(sb, 512)]
                    )
                # V/Wo weights after the round-0 hidden (needed ~10us in)
                nc.sync.dma_start(wv_a, wvt[:, 0:4, :])
                nc.sync.dma_start(wv_b, wvt[:, 4:8, :])
                nc.sync.dma_start(wo_sb, wot)
            else:
                nc.sync.dma_start(hTa, hiddent[:, 0:4, ts(sb, 512)])
                nc.gpsimd.dma_start(hTb, hiddent[:, 4:8, ts(sb, 512)])
            ht_tiles[sb] = (hTa, hTb)

        def hts_of(sb):
            hTa, hTb = ht_tiles[sb]
            return lambda dc: (hTa if dc < 4 else hTb)[:, dc % 4, :]

        def emit_kq(sb, cell=None):
            """K/Q projections for s-block sb, pair-major. When `cell` is
            given (the first cell of round sb), each pair's S-phase is
            emitted right after that pair's two combos, so its exps keep
            ACT fed while the remaining combos run on PE."""
            hts = hts_of(sb)
            pts = []
            for p in range(PAIRS):
                for w_sb, out_t in ((wk_sb, KT), (wq_sb, QT)):
                    kps = fin_ps.tile([P, 512], F32, tag="fin", name="kps")
                    for dc in range(DC):
                        nc.tensor.matmul(
                            kps,
                            w_sb[:, dc, ts(p, P)],
                            hts(dc),
                            start=(dc == 0),
                            stop=(dc == DC - 1),
                        )
                    nc.vector.tensor_copy(out_t[:, p, ts(sb, 512)], kps)
                if cell is not None:
                    pts.append(s_phase(cell[0], cell[1], p))
            if cell is not None:
                pv_queue.append((cell[0], cell[1], pts))

        def emit_vproj(sb, st_i):
            hts = hts_of(sb)
            ki = sb * 4 + st_i
            vps = fin_ps.tile([P, 512], F32, tag="fin", name="vps")
            for dc in range(DC):
                nc.tensor.matmul(
                    vps[:, 0:256],
                    hts(dc)[:, ts(st_i, P)],
                    (wv_a if dc < 4 else wv_b)[:, dc % 4, :],
                    start=(dc == 0),
                    stop=(dc == DC - 1),
                )
            nc.vector.tensor_copy(
                Vp[:, ki, :, 0:64],
                vps[:, 0:256].rearrange("p (h f) -> p h f", h=UNITS),
            )

        issue_dma(0)
        # round 0's K/Q fused with cell (0,0)'s S-phases
        emit_kq(0, cell=(0, 0))
        for sb in range(SB):
            if sb + 1 < SB:
                issue_dma(sb + 1)
            # newly-ready cells: earlier q-blocks against this round's keys,
            # plus this q-block against all keys so far. Cell (0, sb) was
            # already emitted fused into emit_kq(sb).
            new_cells = [(qb, sb) for qb in range(1, sb)]
            new_cells += [(sb, kc) for kc in range(sb + 1)]
            if sb == 0:
                new_cells = []  # (0, 0) already emitted fused in emit_kq(0)
            for st_i in range(4):
                emit_vproj(sb, st_i)
            drain_pv(keep=1)
            for cell in new_cells:
                emit_s(*cell)
                drain_pv(keep=1)
            # next round's K/Q (+ its first cell) at the round's lowest
            # priority: PE falls back to them when the cell stream stalls
            if sb + 1 < SB:
                emit_kq(sb + 1, cell=(0, sb + 1))
        drain_pv(keep=0)
    nc.compile()
    return nc


_NC = None


def get_nc():
    global _NC
    if _NC is None:
        _NC = build_nc()
    return _NC


def shard_inputs(hidden_states, Wq, Wk, Wv, Wo):
    """Per-core input maps. Core c: batch c//4, heads 4*(c%4) .. 4*(c%4)+3."""
    import ml_dtypes

    hidden_states = np.asarray(hidden_states, np.float32)
    Wq, Wk, Wv, Wo = (np.asarray(w, np.float32) for w in (Wq, Wk, Wv, Wo))
    in_maps = []
    for c in range(N_CORES):
        b = c // 4
        f0 = (c % 4) * 4 * DIM_HEAD  # first feature row/col of this core's heads
        rows = slice(f0, f0 + UNITS * DIM_HEAD)

        def proj_layout(w):
            # W[rows].T is [D, 256]; on-chip layout is [128, DC, 256] bf16
            return np.ascontiguousarray(
                w[rows, :].T.reshape(DC, P, 256).transpose(1, 0, 2)
            ).astype(ml_dtypes.bfloat16)

        # Wo[:, rows].T is [256, D]; on-chip layout is [128, PAIRS, D] bf16
        wot = np.ascontiguousarray(
            Wo[:, rows].T.reshape(PAIRS, P, D).transpose(1, 0, 2)
        ).astype(ml_dtypes.bfloat16)
        in_maps.append(
            {
                "hiddent": np.ascontiguousarray(
                    hidden_states[b].T.reshape(DC, P, S).transpose(1, 0, 2)
                ).astype(ml_dtypes.bfloat16),
                "wqt": proj_layout(Wq),
                "wkt": proj_layout(Wk),
                "wvt": proj_layout(Wv),
                "wot": wot,
            }
        )
    return in_maps


def unshard_outputs(results, bo):
    out = np.zeros((B, S, D), np.float32)
    for c, res in enumerate(results):
        out[c // 4] += res["y"]
    out += np.asarray(bo, np.float32)[None, None, :]
    return out


def kernel(hidden_states, Wq, Wk, Wv, Wo, bo, _trace=False):
    from concourse.bass_utils import run_bass_kernel_spmd

    nc = get_nc()
    in_maps = shard_inputs(hidden_states, Wq, Wk, Wv, Wo)
    res = run_bass_kernel_spmd(nc, in_maps, list(range(N_CORES)), trace=_trace)
    out = unshard_outputs(res.results, bo)
    if _trace:
        return out, res
    return out


# revision 38
# speedup vs baseline: 1.0561x; 1.0350x over previous
# CrossAttention (B=2, S=2048, D=1024, H=16, dh=64) on 8 trn2 NeuronCores.
#
# Sharding: 32 (batch, head) units, 4 consecutive units per core (cores 0-3
# work on batch 0, cores 4-7 on batch 1). Each core receives its batch's
# hidden states pre-permuted to the on-chip [128, D/128, S] transposed
# layout, per-head slices of Wq/Wk/Wv/Wo (also pre-permuted); it returns a
# partial output y [2048, 1024] (its heads' contribution to the output
# projection). The host sums the four partials per batch and adds bo.
#
# Device algorithm (per core, 4 heads = 2 pairs). The cost model prices a
# matmul at (moving free size) x (cycles/row): fp32r is 1 cyc/row only at
# N>=256, bf16 is 1 cyc/row at any N, and M/K are free. The kernel exploits
# this by keeping the big-N matmuls (projections, scores) in fp32r and
# running PV "swapped" with tiny-N bf16 moving operands:
#   - projections: Q^T/K^T pair-packed [128, s] fp32r (N=512); V in natural
#     [s, feat] layout converted to bf16 with a fused ones column per head.
#   - scores: S^T = K^T-chunk^T Q^T per pair (K=64, M=128 keys, N=512).
#   - P^T = exp(S^T/8) on ACT, written as bf16 (PV stationary).
#   - PV swapped: stationary = P^T tile [128 keys, 128 q], moving =
#     [V_h | 1] bf16 [128 keys, 65] -> psum [128 q, 65] per head; one psum
#     accumulation group per (cell, qtile) over 4 key chunks x 4 heads;
#     column 64 accumulates the softmax denominator. Half the PE cycles of
#     the classic V'^T P^T orientation, and O lands in [q, feat] layout.
#   - normalize: DVE reciprocal of psum col 64, per-partition
#     tensor_scalar_mul -> bf16 O tiles; PE bf16 transpose (identity
#     permutation) -> O^T [feat, q]; output projection with K=128 (head
#     pairs packed) and fp32r moving Wo^T (N=512).
# Triangular software pipeline over 512-wide s-block rounds keeps the ACT
# engine (exp is ~equal to total PE work) fed from round 0.
import os
import sys

import numpy as np

try:
    import concourse.bass as bass
except ImportError:  # harness runs from a fresh dir; repo is on the default path
    sys.path.insert(0, "/opt/trn_rl_repo")
    import concourse.bass as bass

import concourse.bacc as bacc
import concourse.mybir as mybir
import concourse.tile as tile
from concourse.bass import ts, ds
from contextlib import ExitStack

B, S, D = 2, 2048, 1024
HEADS, DIM_HEAD = 16, 64
SCALE = DIM_HEAD**-0.5
N_CORES = 8
UNITS = 4  # heads per core
PAIRS = 2  # head pairs per core
P = 128
SB = S // 512  # 4 s-block rounds (key blocks)
QBN = S // 512  # 4 q-blocks
DC = D // P  # 8 contraction chunks for projections
KI = S // P  # 16 key chunks of 128
F32 = mybir.dt.float32
F32R = mybir.dt.float32r
BF16 = mybir.dt.bfloat16


def build_nc():
    nc = bacc.Bacc("TRN2", target_bir_lowering=False, debug=False)

    hiddent = nc.dram_tensor("hiddent", [P, DC, S], BF16, kind="ExternalInput").ap()
    wqt = nc.dram_tensor("wqt", [P, DC, 256], BF16, kind="ExternalInput").ap()
    wkt = nc.dram_tensor("wkt", [P, DC, 256], BF16, kind="ExternalInput").ap()
    wvt = nc.dram_tensor("wvt", [P, DC, 256], BF16, kind="ExternalInput").ap()
    # Wo^T pair-packed: wot[f, pair, d] = Wo[d, f0 + pair*128 + f]; bf16 so
    # the O-projection (bf16 O^T stationary) has matching input dtypes
    wot = nc.dram_tensor("wot", [P, PAIRS, D], BF16, kind="ExternalInput").ap()
    y = nc.dram_tensor("y", [S, D], F32, kind="ExternalOutput").ap()

    K_PT = int(os.environ.get("K_PT", "36"))
    K_ST = int(os.environ.get("K_ST", "2"))
    K_CELL = int(os.environ.get("K_CELL", "1"))
    K_FIN = int(os.environ.get("K_FIN", "2"))
    K_DEFER = int(os.environ.get("K_DEFER", "0"))
    K_LASTFIRST = os.environ.get("K_LASTFIRST", "0") == "1"

    with tile.TileContext(nc) as tc, ExitStack() as ctx:
        persist = ctx.enter_context(tc.tile_pool(name="persist", bufs=1))
        pt_pool = ctx.enter_context(tc.tile_pool(name="pt", bufs=K_PT))
        otu_pool = ctx.enter_context(tc.tile_pool(name="otu", bufs=6))
        ott_pool = ctx.enter_context(tc.tile_pool(name="ott", bufs=6))
        rc_pool = ctx.enter_context(tc.tile_pool(name="rc", bufs=4))
        y_pool = ctx.enter_context(tc.tile_pool(name="ysb", bufs=4))
        ht_pool = ctx.enter_context(tc.tile_pool(name="htp", bufs=2))
        # PSUM (8 banks): S^T tiles 2x[128,1024] (4 banks), PV accumulators
        # 2x[128,4x65] (2 banks), projection/transpose/output transients
        # 2x[128,512] (2 banks).
        st_ps = ctx.enter_context(
            tc.tile_pool(name="stps", bufs=K_ST, space="PSUM")
        )
        cell_ps = ctx.enter_context(
            tc.tile_pool(name="cellps", bufs=K_CELL, space="PSUM")
        )
        fin_ps = ctx.enter_context(
            tc.tile_pool(name="finps", bufs=K_FIN, space="PSUM")
        )

        # ---- persistent SBUF tensors ----
        KT = persist.tile([P, PAIRS, S], F32R)  # K^T pair-packed
        QT = persist.tile([P, PAIRS, S], F32R)  # Q^T pair-packed
        # V natural layout per (k-chunk, head): [V_h(64) | 1] in bf16
        Vp = persist.tile([P, KI, UNITS, 65], BF16)
        wq_sb = persist.tile([P, DC, 256], BF16)
        wk_sb = persist.tile([P, DC, 256], BF16)
        wv_a = persist.tile([P, 4, 256], BF16)
        wv_b = persist.tile([P, 4, 256], BF16)
        wo_sb = persist.tile([P, PAIRS, D], BF16)  # Wo^T pair-packed (K=128)
        ones_f32 = persist.tile([P, P], F32)
        ident = persist.tile([P, P], BF16)  # identity for PE transpose
        # O accumulator: [q-tile partitions, qtile, head, 64 feats + denom]
        acc = persist.tile([P, KI, UNITS, 65], F32)

        nc.vector.memset(ones_f32, 1.0)
        # identity: keep 1.0 where partition == column, else 0
        ones_bf = persist.tile([P, P], BF16)
        nc.vector.tensor_copy(ones_bf, ones_f32)
        nc.gpsimd.affine_select(
            ident,
            ones_bf,
            pattern=[[-1, P]],
            compare_op=mybir.AluOpType.is_equal,
            fill=0.0,
            base=0,
            channel_multiplier=1,
        )
        # ones columns of V' (col 64 per head); V writes only cols 0:64
        nc.vector.memset(Vp[:, :, :, 64:65], 1.0)
        # warm the ACT Exp table before the first real exp
        warm = persist.tile([P, 1], BF16)
        nc.scalar.activation(
            warm, ones_f32[:, 0:1], mybir.ActivationFunctionType.Exp, scale=1.0
        )
        # warm the PE p-state (2.4GHz needs ~3us of continuous execution):
        # junk matmuls on ones keep the PE busy while the first weight and
        # hidden DMAs land, so the real projections start at full clock
        for w in range(int(os.environ.get("K_WARM", "10"))):
            wps = fin_ps.tile([P, 64], F32, tag="fin", name="wps")
            nc.tensor.matmul(
                wps, ones_f32, ones_f32[:, 0:64], start=True, stop=True
            )
        # first-round weights split across both DMA queues so neither
        # head-blocks: K on HWDGE before the hidden quarters, Q first on
        # SWDGE; V/Wo are emitted after round 0's hidden quarters
        nc.sync.dma_start(wk_sb, wkt)
        nc.gpsimd.dma_start(wq_sb, wqt)

        # ---- cell phases ----
        pv_queue = []  # (qb, kc, pts) with exps emitted, PV pending

        def s_phase(qb, kc, p):
            """S^T + exp for one head pair of cell (qb, kc); returns pt tiles."""
            pts = []
            for k4 in range(4):
                ki = kc * 4 + k4
                stt = st_ps.tile([P, 1024], F32, tag="st")
                nc.tensor.matmul(
                    stt[:, 0:512],
                    KT[0:64, p, ts(ki, P)],
                    QT[0:64, p, ts(qb, 512)],
                    start=True,
                    stop=True,
                )
                nc.tensor.matmul(
                    stt[:, 512:1024],
                    KT[64:128, p, ts(ki, P)],
                    QT[64:128, p, ts(qb, 512)],
                    start=True,
                    stop=True,
                )
                pt = pt_pool.tile([P, 1024], BF16)
                nc.scalar.activation(
                    pt, stt, mybir.ActivationFunctionType.Exp, scale=SCALE
                )
                pts.append(pt)
            return pts

        def pv_phase(qb, kc, pts, y_on_act=False):
            """PV for cell (qb, kc): per q-tile, one psum accumulation group
            over 4 heads x 4 key chunks; accumulate into acc on DVE. On the
            final cell of a q-block, finalize each q-tile as soon as its
            accumulation lands (shortens the post-exp tail)."""
            for qt_i in range(4):
                if os.environ.get("K_CELLFALL", "0") == "1" or (
                    kc == SB - 1 and os.environ.get("K_CELLF", "1") == "1"
                ):
                    # final cells alternate two psum tags so the four
                    # trailing qt chains pipeline two-deep
                    cp = cell_ps.tile(
                        [P, UNITS, 65],
                        F32,
                        tag="cell" if qt_i % 2 == 0 else "cellf",
                        bufs=1,
                        name="cp",
                    )
                else:
                    cp = cell_ps.tile([P, UNITS, 65], F32, tag="cell", bufs=1, name="cp")
                n = 0
                for h in range(UNITS):
                    pr, hh = divmod(h, 2)
                    for k4 in range(4):
                        nc.tensor.matmul(
                            cp[:, h, :],
                            pts[pr][k4][:, hh * 512 + qt_i * P : hh * 512 + (qt_i + 1) * P],
                            Vp[:, kc * 4 + k4, h, :],
                            start=(n == 0),
                            stop=(n == 15),
                        )
                        n += 1
                sl = acc[:, qb * 4 + qt_i, :, :]
                if kc == 0:
                    nc.vector.tensor_copy(sl, cp)
                else:
                    nc.vector.tensor_add(sl, sl, cp)
                if kc == SB - 1:
                    finalize_qt(qb * 4 + qt_i, y_on_act)

        def finalize_qt(qt, y_on_act=False):
            """Normalize q-tile qt, transpose O, output projection, DMA."""
            rc = rc_pool.tile([P, UNITS], F32)
            nc.vector.reciprocal(
                rc, acc[:, qt, :, 64:65].rearrange("p h one -> p (h one)")
            )
            otts = []
            for pr in range(PAIRS):
                otu = otu_pool.tile([P, P], BF16)
                for hh in range(2):
                    h = 2 * pr + hh
                    # all-SBUF op: runs on the otherwise-idle GPSIMD
                    nc.gpsimd.tensor_scalar_mul(
                        otu[:, hh * 64 : (hh + 1) * 64],
                        acc[:, qt, h, 0:64],
                        rc[:, h : h + 1],
                    )
                tp = fin_ps.tile([P, P], BF16, tag="fin", name="tp")
                nc.tensor.transpose(tp, otu, ident)
                ott = ott_pool.tile([P, P], BF16)
                nc.vector.tensor_copy(ott, tp)
                otts.append(ott)
            for dh in range(2):
                yps = fin_ps.tile([P, 512], F32, tag="fin", name="yps")
                for pr in range(PAIRS):
                    nc.tensor.matmul(
                        yps,
                        otts[pr],
                        wo_sb[:, pr, ds(dh * 512, 512)],
                        start=(pr == 0),
                        stop=(pr == PAIRS - 1),
                    )
                ysb = y_pool.tile([P, 512], F32)
                # the tail finalize copies y on ACT (idle once exps drain);
                # earlier ones stay on DVE to keep ACT on the exp stream
                if y_on_act:
                    nc.scalar.copy(ysb, yps)
                else:
                    nc.vector.tensor_copy(ysb, yps)
                nc.sync.dma_start(
                    y[qt * P : (qt + 1) * P, ds(dh * 512, 512)], ysb
                )

        def emit_s(qb, kc):
            pts = [s_phase(qb, kc, 0), s_phase(qb, kc, 1)]
            pv_queue.append((qb, kc, pts))

        n_final = [0]

        def drain_pv(keep, max_kc=SB - 1):
            """Emit pending PVs. max_kc gates cells whose kc-block V
            projection hasn't been emitted yet (tile deps are emission-
            ordered: a PV emitted before its V write would read stale Vp)."""
            while len(pv_queue) > keep and pv_queue[0][1] <= max_kc:
                qb, kc, pts = pv_queue.pop(0)
                if kc == SB - 1:
                    n_final[0] += 1
                pv_phase(
                    qb,
                    kc,
                    pts,
                    y_on_act=(
                        os.environ.get("K_YACT", "1") == "1"
                        and kc == SB - 1
                        and n_final[0] >= QBN
                    ),
                )

        # ---- triangular pipeline ----
        # hidden DMAs and K/Q projections run one round ahead of the cell
        # stream, so each round's cells unlock with no ACT gap; V
        # projections trail the first cell's S-phase (PV needs them ~8us
        # later).
        ht_tiles = {}

        def issue_dma(sb):
            hTa = ht_pool.tile([P, 4, 512], BF16, tag="hta", name="hTa")
            hTb = ht_pool.tile([P, 4, 512], BF16, tag="htb", name="hTb")
            if sb == 0:
                # split the cold-start load into quarters on both queues so
                # the first projection chunks begin as early as possible
                for dq in range(4):
                    nc.sync.dma_start(hTa[:, dq, :], hiddent[:, dq, ts(sb, 512)])
                    nc.gpsimd.dma_start(
                        hTb[:, dq, :], hiddent[:, 4 + dq, ts(sb, 512)]
                    )
                # V/Wo weights after the round-0 hidden (needed ~10us in)
                nc.sync.dma_start(wv_a, wvt[:, 0:4, :])
                nc.sync.dma_start(wv_b, wvt[:, 4:8, :])
                nc.sync.dma_start(wo_sb, wot)
            else:
                nc.sync.dma_start(hTa, hiddent[:, 0:4, ts(sb, 512)])
                nc.gpsimd.dma_start(hTb, hiddent[:, 4:8, ts(sb, 512)])
            ht_tiles[sb] = (hTa, hTb)

        def hts_of(sb):
            hTa, hTb = ht_tiles[sb]
            return lambda dc: (hTa if dc < 4 else hTb)[:, dc % 4, :]

        def emit_kq(sb, cell=None):
            """K/Q projections for s-block sb, pair-major. When `cell` is
            given (the first cell of round sb), each pair's S-phase is
            emitted right after that pair's two combos, so its exps keep
            ACT fed while the remaining combos run on PE."""
            hts = hts_of(sb)
            pts = []
            for p in range(PAIRS):
                for w_sb, out_t in ((wk_sb, KT), (wq_sb, QT)):
                    kps = fin_ps.tile([P, 512], F32, tag="fin", name="kps")
                    for dc in range(DC):
                        nc.tensor.matmul(
                            kps,
                            w_sb[:, dc, ts(p, P)],
                            hts(dc),
                            start=(dc == 0),
                            stop=(dc == DC - 1),
                        )
                    nc.vector.tensor_copy(out_t[:, p, ts(sb, 512)], kps)
                if cell is not None:
                    pts.append(s_phase(cell[0], cell[1], p))
            if cell is not None:
                pv_queue.append((cell[0], cell[1], pts))

        def emit_vproj(sb, st_i):
            hts = hts_of(sb)
            ki = sb * 4 + st_i
            vps = fin_ps.tile([P, 512], F32, tag="fin", name="vps")
            for dc in range(DC):
                nc.tensor.matmul(
                    vps[:, 0:256],
                    hts(dc)[:, ts(st_i, P)],
                    (wv_a if dc < 4 else wv_b)[:, dc % 4, :],
                    start=(dc == 0),
                    stop=(dc == DC - 1),
                )
            nc.vector.tensor_copy(
                Vp[:, ki, :, 0:64],
                vps[:, 0:256].rearrange("p (h f) -> p h f", h=UNITS),
            )

        issue_dma(0)
        # round 0's K/Q fused with cell (0,0)'s S-phases
        emit_kq(0, cell=(0, 0))
        for sb in range(SB):
            if sb + 1 < SB:
                issue_dma(sb + 1)
            # newly-ready cells: earlier q-blocks against this round's keys,
            # plus this q-block against all keys so far. Cell (0, sb) was
            # already emitted fused into emit_kq(sb).
            new_cells = [(qb, sb) for qb in range(1, sb)]
            new_cells += [(sb, kc) for kc in range(sb + 1)]
            if sb == 0:
                new_cells = []  # (0, 0) already emitted fused in emit_kq(0)
            for cell in new_cells:
                emit_s(*cell)
                # PVs of kc==sb cells wait until V_sb is emitted below
                drain_pv(keep=1, max_kc=sb - 1)
            # next round's K/Q (+ its first cell) ahead of this round's V
            # projections: the next cell's exps flow with no round gap
            if sb + 1 < SB:
                emit_kq(sb + 1, cell=(0, sb + 1))
            for st_i in range(4):
                emit_vproj(sb, st_i)
            drain_pv(keep=1)
        drain_pv(keep=0)
    nc.compile()
    return nc


_NC = None


def get_nc():
    global _NC
    if _NC is None:
        _NC = build_nc()
    return _NC


def shard_inputs(hidden_states, Wq, Wk, Wv, Wo):
    """Per-core input maps. Core c: batch c//4, heads 4*(c%4) .. 4*(c%4)+3."""
    import ml_dtypes

    hidden_states = np.asarray(hidden_states, np.float32)
    Wq, Wk, Wv, Wo = (np.asarray(w, np.float32) for w in (Wq, Wk, Wv, Wo))
    in_maps = []
    for c in range(N_CORES):
        b = c // 4
        f0 = (c % 4) * 4 * DIM_HEAD  # first feature row/col of this core's heads
        rows = slice(f0, f0 + UNITS * DIM_HEAD)

        def proj_layout(w):
            # W[rows].T is [D, 256]; on-chip layout is [128, DC, 256] bf16
            return np.ascontiguousarray(
                w[rows, :].T.reshape(DC, P, 256).transpose(1, 0, 2)
            ).astype(ml_dtypes.bfloat16)

        # Wo[:, rows].T is [256, D]; on-chip layout is [128, PAIRS, D] bf16
        wot = np.ascontiguousarray(
            Wo[:, rows].T.reshape(PAIRS, P, D).transpose(1, 0, 2)
        ).astype(ml_dtypes.bfloat16)
        in_maps.append(
            {
                "hiddent": np.ascontiguousarray(
                    hidden_states[b].T.reshape(DC, P, S).transpose(1, 0, 2)
                ).astype(ml_dtypes.bfloat16),
                "wqt": proj_layout(Wq),
                "wkt": proj_layout(Wk),
                "wvt": proj_layout(Wv),
                "wot": wot,
            }
        )
    return in_maps


def unshard_outputs(results, bo):
    out = np.zeros((B, S, D), np.float32)
    for c, res in enumerate(results):
        out[c // 4] += res["y"]
    out += np.asarray(bo, np.float32)[None, None, :]
    return out


def kernel(hidden_states, Wq, Wk, Wv, Wo, bo, _trace=False):
    from concourse.bass_utils import run_bass_kernel_spmd

    nc = get_nc()
    in_maps = shard_inputs(hidden_states, Wq, Wk, Wv, Wo)
    res = run_bass_kernel_spmd(nc, in_maps, list(range(N_CORES)), trace=_trace)
    out = unshard_outputs(res.results, bo)
    if _trace:
        return out, res
    return out
